# revision 12
# baseline (speedup 1.0000x reference)
"""FFT spatially-variant blur via a rank-3 linear-in-coc factorization.

Reference math: out = sum_k wbar_k(coc) * (psf_k (*) x), with mixture
weights wbar_k over 8 Gaussian PSF bases, sigma = clip(softplus(
0.3*coc + 0.5), 0.2, 12).  With coc in [0,1), sigma lies in
[0.974, 1.172]: the per-pixel effective kernel field is fit as

    K(c) ~= P0 + c * P1,   P0 rank-2, P1 rank-1  (field rel ~6.5e-3)

so the module becomes THREE separable convolutions (r=0,1 -> plane A;
r=2 -> plane B) plus a fused per-pixel mix  out = A + coc .* B.

Device schedule (per core = one batch sample, 3 channels):

  stage 1 (column conv, image stationary):  per (ch, col-tile mt):
    FOUR matmuls, one per row-chunk q: rhs = compact r-packed band
    table t1c[q] [128, 3*160], dst = its own PSUM bank [0,480).
    Band seams between adjacent q are resolved in the DRAIN: 4 region
    copies (ACT) + 3 seam adds (DVE) assemble ab[ch,mt][128, 3*512]
    bf16.  48 matmuls total (vs 120 for windowed accumulation), and
    the compact tables need no zero-padding DMAs or memsets.

  stage 2 (row conv, ab stationary, natural orientation): per
    (ch, row-tile mtc): two PSUM banks (planes A, B).  Each bank is
    opened by ONE full-width N=512 matmul whose rhs is the zero-padded
    q2=0 band row (start=True resets the whole bank), then compact
    N=160 band matmuls accumulate the remaining (q2, r) contributions
    -- overlapping dst regions simply accumulate in PSUM, so no seam
    handling at all.  Out partition = image row: coc and the output
    stay in natural orientation (no host transposes).

  mix: m = B .* coc (Pool engine, f32) ; acc = A + m (DVE, -> bf16)
  DMA: contiguous compact tables; issues spread over the sync/ACT/DVE/
    Pool queues (Pool issue is ~25ns); outputs issued from Pool.

Measured end-to-end rel err ~7e-3 vs the 2e-2 gate (bf16 + fit).

Data parallel: core b handles batch sample b (3 channels each).
"""

import numpy as np
import ml_dtypes

PSF_SIZE = 31
SIGMA_MIN = 0.2
SIGMA_MAX = 12.0
EPS = 1e-9
H = 512
NCHUNK = 4   # 512 / 128
R = 3        # separable filters: 0,1 -> plane A; 2 -> plane B
_PLANE_RS = [(0, 1), (2,)]
_OPENER_RS = [0, 2]   # plane-lead filters: opener rows of t2open

# compact band column ranges per chunk (width 160 covers the 158-wide band)
_BAND_C0 = [0, 113, 241, 352]
_BAND_W = 160

# stage-1 band seams are accumulated in PSUM: chunk q>0 emits a small
# seam matmul (first 30 band cols, accumulated into chunk q-1's bank)
# and a main matmul (cols 30:160 into its own bank), so the drain is
# four pure region copies: (q, c0, c1, src col offset in bank q).
_COPIES = [(0, 0, 143, 0), (1, 143, 271, 30), (2, 271, 399, 30),
           (3, 399, 512, 47)]
# seam q: (src col in q's own band table, dst col in bank q-1's layout)
# -- q=3's band starts at table col 17 (C0=352, band start 369)
_SEAM_J0 = {1: 0, 2: 0, 3: 17}
_SEAM_OFF = {1: 113, 2: 128, 3: 128}


def _filters(ws, bs):
    """Rank-(2+1) linear-in-c factorization of the kernel field via
    alternating least squares: K(c) ~= P0 + c*P1 with P0 rank-2 and
    P1 rank-1.

    Returns (t1_taps[3][31], t2_taps[3][31]) fp64; filter r contributes
    outer(t1[r], t2[r]) to plane A (r<2) or plane B (r=2)."""
    lo = (-PSF_SIZE) // 2
    hi = PSF_SIZE // 2
    x = np.linspace(lo, hi, PSF_SIZE, dtype=np.float32).astype(np.float64)
    gx, gy = np.meshgrid(x, x, indexing='ij')
    sigmas = np.linspace(SIGMA_MIN, SIGMA_MAX, 8, dtype=np.float32)
    sigmas = sigmas.astype(np.float64)
    psfs = []
    for s in sigmas:
        g = np.exp(-(gx ** 2 + gy ** 2) / (2.0 * s * s + EPS))
        psfs.append(g / (g.sum() + EPS))
    psfs = np.array(psfs).reshape(8, -1)

    cg = np.linspace(0.0, 1.0, 2001)
    sig = np.clip(np.logaddexp(0.0, ws * cg + bs), SIGMA_MIN, SIGMA_MAX)
    w = np.exp(-(sig[:, None] - sigmas[None, :]) ** 2 / 2.0)
    w = w / (w.sum(1, keepdims=True) + EPS)
    M = w @ psfs                                     # [nc, 961]
    V = np.vander(cg, 2, increasing=True)            # [nc, 2]

    def proj(P, rank):
        evals, evecs = np.linalg.eigh(P.reshape(PSF_SIZE, PSF_SIZE))
        idx = np.argsort(-np.abs(evals))[:rank]
        flat = sum(evals[i] * np.outer(evecs[:, i], evecs[:, i])
                   for i in idx).reshape(-1)
        return flat, [(evals[i], evecs[:, i]) for i in idx]

    coef, *_ = np.linalg.lstsq(V, M, rcond=None)
    P0, P1 = coef[0], coef[1]
    for _ in range(200):
        P0r, _f = proj(P0, 2)
        P1f, *_ = np.linalg.lstsq(V[:, 1:2], M - V[:, 0:1] @ P0r[None, :],
                                  rcond=None)
        P1r, _f = proj(P1f[0], 1)
        P0f, *_ = np.linalg.lstsq(V[:, 0:1], M - V[:, 1:2] @ P1r[None, :],
                                  rcond=None)
        P0, P1 = P0f[0], P1f[0]
    _, f0 = proj(P0, 2)
    _, f1 = proj(P1, 1)
    t1, t2 = [], []
    for lam, u in f0 + f1:
        t1.append(u)
        t2.append(lam * u)
    return t1, t2


def _band_tables(taps_list):
    """Compact band tables [4 (q), 128, R*160] bf16:
    tab[q][p, r*160 + (c - c0q)] = taps_r[15 + c - (128q+p)]."""
    tab = np.zeros((NCHUNK, 128, R * _BAND_W), dtype=np.float64)
    for r, taps in enumerate(taps_list):
        for q in range(NCHUNK):
            c0 = _BAND_C0[q]
            for p in range(128):
                row = 128 * q + p
                j0 = max(c0, row - 15)
                j1 = min(c0 + _BAND_W, row + 16, H)
                if j1 > j0:
                    tab[q, p, r * _BAND_W + j0 - c0:
                        r * _BAND_W + j1 - c0] = \
                        taps[15 + np.arange(j0, j1) - row]
    return tab.astype(ml_dtypes.bfloat16)


def _opener_table(taps_list):
    """Full-width zero-padded q=0 band rows for the plane-opening
    matmuls: open[p, i, c'] = taps_{OPENER_RS[i]}[15 + c' - p]."""
    open_ = np.zeros((128, len(_OPENER_RS), H), dtype=np.float64)
    for i, r in enumerate(_OPENER_RS):
        taps = taps_list[r]
        for p in range(128):
            j0 = max(0, p - 15)
            j1 = min(H, p + 16)
            open_[p, i, j0:j1] = taps[15 + np.arange(j0, j1) - p]
    return open_.astype(ml_dtypes.bfloat16)


def _build():
    import concourse.bass as bass  # noqa: F401
    import concourse.tile as tile
    from concourse import mybir, bacc

    f32 = mybir.dt.float32
    bf16 = mybir.dt.bfloat16
    AF = mybir.ActivationFunctionType
    ALU = mybir.AluOpType

    nc = bacc.Bacc("TRN2", target_bir_lowering=False, debug=False,
                   disable_frame_to_traceback=True)
    IMG = nc.declare_dram_parameter("image", [3, H, H], bf16, isOutput=False)
    COC = nc.declare_dram_parameter("coc32", [H, H], f32, isOutput=False)
    T1C = nc.declare_dram_parameter("t1c", [NCHUNK, 128, R * _BAND_W], bf16,
                                    isOutput=False)
    T2C = nc.declare_dram_parameter("t2c", [NCHUNK, 128, R * _BAND_W], bf16,
                                    isOutput=False)
    T2O = nc.declare_dram_parameter("t2open", [128, len(_OPENER_RS), H],
                                    bf16, isOutput=False)
    OUT = nc.declare_dram_parameter("out", [3, H, H], bf16, isOutput=True)

    BW = _BAND_W

    with tile.TileContext(nc) as tc:
        import contextlib
        ctx = contextlib.ExitStack()
        with ctx:
            tpool = ctx.enter_context(tc.tile_pool(name="ttab", bufs=1))
            cpool = ctx.enter_context(tc.tile_pool(name="coc", bufs=1))
            xpool = ctx.enter_context(tc.tile_pool(name="xin", bufs=1))
            apool = ctx.enter_context(tc.tile_pool(name="abig", bufs=8))
            mpool = ctx.enter_context(tc.tile_pool(name="mtmp", bufs=3))
            accpool = ctx.enter_context(tc.tile_pool(name="acc", bufs=3))
            ps = ctx.enter_context(
                tc.tile_pool(name="ps", bufs=8, space="PSUM"))

            t1c = tpool.tile([128, NCHUNK * R * BW], bf16, tag="t1c")
            t2c = tpool.tile([128, NCHUNK * R * BW], bf16, tag="t2c")
            t2open = tpool.tile([128, len(_OPENER_RS) * H], bf16, tag="t2o")
            xs = [xpool.tile([128, NCHUNK * H], bf16, tag=f"xs{ch}",
                             name=f"xs{ch}")
                  for ch in range(3)]
            coc = cpool.tile([128, NCHUNK * H], f32, tag="coc")

            # --- input DMAs, spread across queues so issue cost never
            # serializes and the first stage-1 matmul gates on only
            # t1c[q0] + image chunk 0.
            # sync queue: t1 tables q-interleaved for earliest gating
            for q in range(NCHUNK):
                nc.sync.dma_start(t1c[:, q * R * BW:(q + 1) * R * BW],
                                  T1C[q])
            # scalar(ACT) queue: ch0 image chunks
            for q in range(NCHUNK):
                nc.scalar.dma_start(xs[0][:, q * H:(q + 1) * H],
                                    IMG[0][128 * q:128 * (q + 1), :])
            # sync queue (behind the tables): ch1 image chunks
            for q in range(NCHUNK):
                nc.sync.dma_start(xs[1][:, q * H:(q + 1) * H],
                                  IMG[1][128 * q:128 * (q + 1), :])
            # pool queue (cheap issues): stage-2 tables, coc, ch2 image
            nc.gpsimd.dma_start(
                t2c[:].rearrange("p (q j) -> p q j", q=NCHUNK),
                T2C.rearrange("q p j -> p q j"))
            nc.gpsimd.dma_start(t2open[:],
                                T2O.rearrange("p i j -> p (i j)"))
            nc.gpsimd.dma_start(
                coc[:].rearrange("p (q j) -> p q j", q=NCHUNK),
                COC.rearrange("(q p) j -> p q j", p=128))
            for q in range(NCHUNK):
                nc.gpsimd.dma_start(xs[2][:, q * H:(q + 1) * H],
                                    IMG[2][128 * q:128 * (q + 1), :])

            blkno = [0]

            def emit_stage1(ch, mt):
                """Column conv block: ab[p=col, r, c] = CC_r[c, 128mt+p].
                One matmul per row-chunk q into its own bank; band seams
                between adjacent q resolved in the drain."""
                banks = [ps.tile([128, 512], f32, tag="ps",
                                 name=f"b1_{ch}_{mt}_{q}")
                         for q in range(NCHUNK)]

                def bcols(q, j0, j1):
                    return banks[q][:, 0:R * BW].rearrange(
                        "p (r j) -> p r j", r=R)[:, :, j0:j1]

                def tcols(q, j0, j1):
                    return t1c[:, q * R * BW:(q + 1) * R * BW].rearrange(
                        "p (r j) -> p r j", r=R)[:, :, j0:j1]

                for q in range(NCHUNK):
                    lhsT = xs[ch][:, q * H + 128 * mt: q * H + 128 * mt + 128]
                    if q == 0:
                        nc.tensor.matmul(banks[0][:, 0:R * BW], lhsT,
                                         t1c[:, 0:R * BW],
                                         start=True, stop=False,
                                         skip_group_check=True)
                    else:
                        so = _SEAM_OFF[q]
                        j0 = _SEAM_J0[q]
                        nc.tensor.matmul(bcols(q - 1, so, so + 30), lhsT,
                                         tcols(q, j0, j0 + 30),
                                         start=False, stop=True,
                                         skip_group_check=True)
                        nc.tensor.matmul(bcols(q, j0 + 30, BW), lhsT,
                                         tcols(q, j0 + 30, BW),
                                         start=True,
                                         stop=(q == NCHUNK - 1),
                                         skip_group_check=True)
                ab = apool.tile([128, R * H], bf16, tag="ab",
                                name=f"ab{ch}_{mt}")

                def aview(c0, c1):
                    return ab[:].rearrange(
                        "p (r c) -> p r c", r=R)[:, :, c0:c1]

                # drains: four pure region copies, one on DVE, rest ACT
                for ci, (q, c0, c1, j0) in enumerate(_COPIES):
                    src = bcols(q, j0, j0 + (c1 - c0))
                    dst = aview(c0, c1)
                    if ci == 3:
                        nc.vector.tensor_copy(dst, src)
                    else:
                        nc.scalar.activation(dst, src, AF.Copy)
                return ab

            def emit_s2_mix(ch, abig, mtc):
                """Row conv + mix, natural orientation: out[c, c'] for
                rows c in tile mtc.  Per plane one PSUM bank: a full
                width opener matmul (q2=0 lead filter, zero-padded rhs)
                resets the bank, compact band matmuls accumulate the
                rest; overlapping dst regions accumulate in PSUM."""
                planes = []
                for pl, rs in enumerate(_PLANE_RS):
                    zb = ps.tile([128, 512], f32, tag="ps",
                                 name=f"z{pl}_{ch}_{mtc}")
                    mms = []
                    for q2 in range(NCHUNK):
                        for r in rs:
                            mms.append((q2, r))
                    last = mms[-1]
                    for q2, r in mms:
                        lhsT = abig[q2][:, r * H + 128 * mtc:
                                        r * H + 128 * mtc + 128]
                        if q2 == 0 and r == rs[0]:
                            oi = _OPENER_RS.index(r)
                            rhs = t2open[:, oi * H:(oi + 1) * H]
                            nc.tensor.matmul(zb[:], lhsT, rhs,
                                             start=True,
                                             stop=((q2, r) == last))
                        else:
                            rhs = t2c[:, q2 * R * BW + r * BW:
                                      q2 * R * BW + (r + 1) * BW]
                            c0 = _BAND_C0[q2]
                            nc.tensor.matmul(zb[:, c0:c0 + BW], lhsT, rhs,
                                             start=False,
                                             stop=((q2, r) == last))
                    planes.append(zb)
                csl = coc[:, mtc * H:(mtc + 1) * H]
                m = mpool.tile([128, 512], f32, tag="m")
                nc.vector.tensor_tensor(m[:], planes[1][:], csl, ALU.mult)
                acc = accpool.tile([128, 512], bf16, tag="acc")
                nc.vector.tensor_tensor(acc[:], planes[0][:], m[:],
                                        ALU.add)
                nc.gpsimd.dma_start(
                    OUT[ch][128 * mtc:128 * (mtc + 1), :], acc[:])

            # software pipeline: stage-2 of channel k interleaves with
            # stage-1 of channel k+1 so the PE never drains
            ab0 = [emit_stage1(0, mt) for mt in range(NCHUNK)]
            ab1 = []
            ab1.append(emit_stage1(1, 0))
            emit_s2_mix(0, ab0, 0)
            ab1.append(emit_stage1(1, 1))
            emit_s2_mix(0, ab0, 1)
            ab1.append(emit_stage1(1, 2))
            emit_s2_mix(0, ab0, 2)
            ab1.append(emit_stage1(1, 3))
            emit_s2_mix(0, ab0, 3)
            ab2 = []
            ab2.append(emit_stage1(2, 0))
            emit_s2_mix(1, ab1, 0)
            ab2.append(emit_stage1(2, 1))
            emit_s2_mix(1, ab1, 1)
            ab2.append(emit_stage1(2, 2))
            emit_s2_mix(1, ab1, 2)
            ab2.append(emit_stage1(2, 3))
            emit_s2_mix(1, ab1, 3)
            emit_s2_mix(2, ab2, 0)
            emit_s2_mix(2, ab2, 1)
            emit_s2_mix(2, ab2, 2)
            emit_s2_mix(2, ab2, 3)

    nc.compile()
    return nc


_PROG = None


def _get_prog():
    global _PROG
    if _PROG is None:
        _PROG = _build()
    return _PROG


_TABLES = {}


def _get_tables(ws, bs):
    key = (float(ws), float(bs))
    if key not in _TABLES:
        t1, t2 = _filters(*key)
        _TABLES[key] = (_band_tables(t1), _band_tables(t2),
                        _opener_table(t2))
    return _TABLES[key]


def make_in_maps(image, coc_map, w_sigma, b_sigma):
    bf = ml_dtypes.bfloat16
    tab1, tab2, t2open = _get_tables(
        float(np.asarray(w_sigma).reshape(-1)[0]),
        float(np.asarray(b_sigma).reshape(-1)[0]))
    image = np.asarray(image)
    coc_map = np.asarray(coc_map)
    in_maps = []
    for b in range(image.shape[0]):
        in_maps.append({
            "image": np.ascontiguousarray(image[b].astype(bf)),
            "coc32": np.ascontiguousarray(
                coc_map[b, 0].astype(np.float32)),
            "t1c": tab1,
            "t2c": tab2,
            "t2open": t2open,
        })
    return in_maps


def kernel(image, coc_map, psf_params, w_sigma, b_sigma):
    from concourse.bass_utils import run_bass_kernel_spmd

    B = image.shape[0]
    assert image.shape == (8, 3, H, H)
    nc = _get_prog()
    in_maps = make_in_maps(image, coc_map, w_sigma, b_sigma)
    res = run_bass_kernel_spmd(nc, in_maps, core_ids=list(range(B)))
    out = np.stack([res.results[b]["out"] for b in range(B)], axis=0)
    return np.ascontiguousarray(out).astype(np.float32)


if __name__ == "__main__":
    _get_prog()
    print("build ok")


# revision 13
# speedup vs baseline: 1.0147x; 1.0147x over previous
"""FFT spatially-variant blur via a rank-3 linear-in-coc factorization.

Reference math: out = sum_k wbar_k(coc) * (psf_k (*) x), with mixture
weights wbar_k over 8 Gaussian PSF bases, sigma = clip(softplus(
0.3*coc + 0.5), 0.2, 12).  With coc in [0,1), sigma lies in
[0.974, 1.172]: the per-pixel effective kernel field is fit as

    K(c) ~= P0 + c * P1,   P0 rank-2, P1 rank-1  (field rel ~6.5e-3)

so the module becomes THREE separable convolutions (r=0,1 -> plane A;
r=2 -> plane B) plus a fused per-pixel mix  out = A + coc .* B.

Device schedule (per core = one batch sample, 3 channels):

  stage 1 (column conv, image stationary): per (ch, col-tile mt) one
    4-bank PSUM tile; bank q holds band cols of row-chunk q with the
    128-aligned layout C0[q] = 128q-16, so bank q's cols [16,144)
    are exactly output rows [128q, 128q+128).  Ten matmuls: four
    N=384 mains plus six N=45 seam matmuls that accumulate the
    cross-chunk band overlap straight into the neighbor bank, so the
    drain is ONE 4D copy [128, 3r, 4q, 128] -> ab bf16 per block.

  stage 2 (row conv, ab stationary, natural orientation): per
    (ch, mtc-PAIR) one 4-bank PSUM tile [A0 A1 B0 B1].  Each half is
    opened by a full-width N=512 matmul (zero-padded q2=0 band row,
    start=True resets the bank) and compact N=160 band matmuls
    accumulate the remaining (q2, r) contributions -- overlapping dst
    regions accumulate in PSUM.  The mix then runs at pair width:
    m = Bpair .* coc (DVE), acc = Apair + m (DVE, bf16), one 256-row
    output DMA.  Out partition = image row, so coc and the output
    stay in natural orientation (no host transposes).

Measured end-to-end rel err ~7e-3 vs the 2e-2 gate (bf16 + fit).

Data parallel: core b handles batch sample b (3 channels each).
"""

import numpy as np
import ml_dtypes

PSF_SIZE = 31
SIGMA_MIN = 0.2
SIGMA_MAX = 12.0
EPS = 1e-9
H = 512
NCHUNK = 4   # 512 / 128
R = 3        # separable filters: 0,1 -> plane A; 2 -> plane B
_PLANE_RS = [(0, 1), (2,)]
_OPENER_RS = [0, 2]   # plane-lead filters: opener rows of t2open
BW = 160

# stage-1 band layout: bank q covers output rows [128q-16, 128q+144)
_C0_S1 = [128 * q - 16 for q in range(NCHUNK)]
# stage-2 band layout: compact dst regions inside the 512-wide bank
_C0_S2 = [0, 113, 241, 352]


def _filters(ws, bs):
    """Rank-(2+1) linear-in-c factorization of the kernel field via
    alternating least squares: K(c) ~= P0 + c*P1 with P0 rank-2 and
    P1 rank-1.

    Returns (t1_taps[3][31], t2_taps[3][31]) fp64; filter r contributes
    outer(t1[r], t2[r]) to plane A (r<2) or plane B (r=2)."""
    lo = (-PSF_SIZE) // 2
    hi = PSF_SIZE // 2
    x = np.linspace(lo, hi, PSF_SIZE, dtype=np.float32).astype(np.float64)
    gx, gy = np.meshgrid(x, x, indexing='ij')
    sigmas = np.linspace(SIGMA_MIN, SIGMA_MAX, 8, dtype=np.float32)
    sigmas = sigmas.astype(np.float64)
    psfs = []
    for s in sigmas:
        g = np.exp(-(gx ** 2 + gy ** 2) / (2.0 * s * s + EPS))
        psfs.append(g / (g.sum() + EPS))
    psfs = np.array(psfs).reshape(8, -1)

    cg = np.linspace(0.0, 1.0, 2001)
    sig = np.clip(np.logaddexp(0.0, ws * cg + bs), SIGMA_MIN, SIGMA_MAX)
    w = np.exp(-(sig[:, None] - sigmas[None, :]) ** 2 / 2.0)
    w = w / (w.sum(1, keepdims=True) + EPS)
    M = w @ psfs                                     # [nc, 961]
    V = np.vander(cg, 2, increasing=True)            # [nc, 2]

    def proj(P, rank):
        evals, evecs = np.linalg.eigh(P.reshape(PSF_SIZE, PSF_SIZE))
        idx = np.argsort(-np.abs(evals))[:rank]
        flat = sum(evals[i] * np.outer(evecs[:, i], evecs[:, i])
                   for i in idx).reshape(-1)
        return flat, [(evals[i], evecs[:, i]) for i in idx]

    coef, *_ = np.linalg.lstsq(V, M, rcond=None)
    P0, P1 = coef[0], coef[1]
    for _ in range(200):
        P0r, _f = proj(P0, 2)
        P1f, *_ = np.linalg.lstsq(V[:, 1:2], M - V[:, 0:1] @ P0r[None, :],
                                  rcond=None)
        P1r, _f = proj(P1f[0], 1)
        P0f, *_ = np.linalg.lstsq(V[:, 0:1], M - V[:, 1:2] @ P1r[None, :],
                                  rcond=None)
        P0, P1 = P0f[0], P1f[0]
    _, f0 = proj(P0, 2)
    _, f1 = proj(P1, 1)
    t1, t2 = [], []
    for lam, u in f0 + f1:
        t1.append(u)
        t2.append(lam * u)
    return t1, t2


def _band_tables(taps_list, c0s):
    """Compact band tables [4 (q), 128, R*160] bf16:
    tab[q][p, r*160 + (c - c0s[q])] = taps_r[15 + c - (128q+p)]."""
    tab = np.zeros((NCHUNK, 128, R * BW), dtype=np.float64)
    for r, taps in enumerate(taps_list):
        for q in range(NCHUNK):
            c0 = c0s[q]
            for p in range(128):
                row = 128 * q + p
                j0 = max(c0, row - 15, 0)
                j1 = min(c0 + BW, row + 16, H)
                if j1 > j0:
                    tab[q, p, r * BW + j0 - c0:
                        r * BW + j1 - c0] = \
                        taps[15 + np.arange(j0, j1) - row]
    return tab.astype(ml_dtypes.bfloat16)


def _opener_table(taps_list):
    """Full-width zero-padded q=0 band rows for the plane-opening
    matmuls: open[p, i, c'] = taps_{OPENER_RS[i]}[15 + c' - p]."""
    open_ = np.zeros((128, len(_OPENER_RS), H), dtype=np.float64)
    for i, r in enumerate(_OPENER_RS):
        taps = taps_list[r]
        for p in range(128):
            j0 = max(0, p - 15)
            j1 = min(H, p + 16)
            open_[p, i, j0:j1] = taps[15 + np.arange(j0, j1) - p]
    return open_.astype(ml_dtypes.bfloat16)


def _build():
    import concourse.bass as bass  # noqa: F401
    import concourse.tile as tile
    from concourse import mybir, bacc

    f32 = mybir.dt.float32
    bf16 = mybir.dt.bfloat16
    AF = mybir.ActivationFunctionType
    ALU = mybir.AluOpType

    nc = bacc.Bacc("TRN2", target_bir_lowering=False, debug=False,
                   disable_frame_to_traceback=True)
    IMG = nc.declare_dram_parameter("image", [3, H, H], bf16, isOutput=False)
    COC = nc.declare_dram_parameter("coc", [H, H], bf16, isOutput=False)
    T1C = nc.declare_dram_parameter("t1c", [NCHUNK, 128, R * BW], bf16,
                                    isOutput=False)
    T2C = nc.declare_dram_parameter("t2c", [NCHUNK, 128, R * BW], bf16,
                                    isOutput=False)
    T2O = nc.declare_dram_parameter("t2open", [128, len(_OPENER_RS), H],
                                    bf16, isOutput=False)
    OUT = nc.declare_dram_parameter("out", [3, H, H], bf16, isOutput=True)

    with tile.TileContext(nc) as tc:
        import contextlib
        ctx = contextlib.ExitStack()
        with ctx:
            tpool = ctx.enter_context(tc.tile_pool(name="ttab", bufs=1))
            cpool = ctx.enter_context(tc.tile_pool(name="coc", bufs=1))
            xpool = ctx.enter_context(tc.tile_pool(name="xin", bufs=1))
            apool = ctx.enter_context(tc.tile_pool(name="abig", bufs=12))
            mpool = ctx.enter_context(tc.tile_pool(name="mtmp", bufs=2))
            accpool = ctx.enter_context(tc.tile_pool(name="acc", bufs=2))
            ps = ctx.enter_context(
                tc.tile_pool(name="ps", bufs=2, space="PSUM"))

            t1c = tpool.tile([128, NCHUNK * R * BW], bf16, tag="t1c")
            t2c = tpool.tile([128, NCHUNK * R * BW], bf16, tag="t2c")
            t2open = tpool.tile([128, len(_OPENER_RS) * H], bf16, tag="t2o")
            xs = [xpool.tile([128, NCHUNK * H], bf16, tag=f"xs{ch}",
                             name=f"xs{ch}")
                  for ch in range(3)]
            coc = cpool.tile([128, NCHUNK * H], bf16, tag="coc")

            # --- input DMAs.  The first stage-1 matmul gates on only
            # t1c[q0] + the first image column block; queue assignment
            # keeps the scalar queue free for drains after its 5 issues.
            for q in range(NCHUNK):
                nc.sync.dma_start(t1c[:, q * R * BW:(q + 1) * R * BW],
                                  T1C[q])
            nc.scalar.dma_start(xs[0][:, 0:128], IMG[0][0:128, 0:128])
            nc.scalar.dma_start(xs[0][:, 128:H], IMG[0][0:128, 128:])
            for q in range(1, NCHUNK):
                nc.scalar.dma_start(xs[0][:, q * H:(q + 1) * H],
                                    IMG[0][128 * q:128 * (q + 1), :])
            for q in range(NCHUNK):
                nc.sync.dma_start(xs[1][:, q * H:(q + 1) * H],
                                  IMG[1][128 * q:128 * (q + 1), :])
            for q in range(NCHUNK):
                nc.sync.dma_start(xs[2][:, q * H:(q + 1) * H],
                                  IMG[2][128 * q:128 * (q + 1), :])
            nc.gpsimd.dma_start(
                t2c[:].rearrange("p (q j) -> p q j", q=NCHUNK),
                T2C.rearrange("q p j -> p q j"))
            nc.gpsimd.dma_start(t2open[:],
                                T2O.rearrange("p i j -> p (i j)"))
            nc.gpsimd.dma_start(
                coc[:].rearrange("p (q j) -> p q j", q=NCHUNK),
                COC.rearrange("(q p) j -> p q j", p=128))

            def emit_stage1(ch, mt, drain_dve):
                """Column conv block: ab[p=col, r, c] = CC_r[c, 128mt+p].
                One 4-bank PSUM tile; bank q = 128-aligned band of row
                chunk q; cross-chunk seams accumulate into the neighbor
                bank via N=45 matmuls; drain is one 4D copy."""
                P = ps.tile([128, NCHUNK * 512], f32, tag="ps",
                            name=f"b1_{ch}_{mt}")

                def pview(q, j0, j1):
                    return P[:, q * 512: q * 512 + R * BW].rearrange(
                        "p (r j) -> p r j", r=R)[:, :, j0:j1]

                def tview(q, j0, j1):
                    return t1c[:, q * R * BW:(q + 1) * R * BW].rearrange(
                        "p (r j) -> p r j", r=R)[:, :, j0:j1]

                def lhs(q):
                    return xs[ch][:, q * H + 128 * mt:
                                  q * H + 128 * mt + 128]

                # mains: bank q <- own rows, cols [16,144)
                for q in range(NCHUNK):
                    nc.tensor.matmul(pview(q, 16, 144), lhs(q),
                                     tview(q, 16, 144),
                                     start=True, stop=False,
                                     skip_group_check=True)
                # seams: chunk s -> bank s-1 cols [129,144) (its table
                # cols [1,16)) and bank s+1 cols [16,31) (cols [144,159))
                last_for_bank = {0: (1, 'dn'), 1: (2, 'dn'), 2: (3, 'dn'),
                                 3: (2, 'up')}
                for s in range(NCHUNK):
                    if s >= 1:
                        stop = last_for_bank[s - 1] == (s, 'dn')
                        nc.tensor.matmul(pview(s - 1, 129, 144), lhs(s),
                                         tview(s, 1, 16),
                                         start=False, stop=stop,
                                         skip_group_check=True)
                    if s <= NCHUNK - 2:
                        stop = last_for_bank[s + 1] == (s, 'up')
                        nc.tensor.matmul(pview(s + 1, 16, 31), lhs(s),
                                         tview(s, 144, 159),
                                         start=False, stop=stop,
                                         skip_group_check=True)
                ab = apool.tile([128, R * H], bf16, tag="ab",
                                name=f"ab{ch}_{mt}")
                # single 4D drain: [p, r, q, j(128)] from cols [16,144)
                src = P[:].rearrange("p (q x) -> p q x", q=NCHUNK)[
                    :, :, 0:R * BW].rearrange(
                    "p q (r j) -> p r q j", r=R)[:, :, :, 16:144]
                dst = ab[:].rearrange("p (r q j) -> p r q j",
                                      q=NCHUNK, j=128)
                if drain_dve:
                    nc.vector.tensor_copy(dst, src)
                else:
                    nc.scalar.activation(dst, src, AF.Copy)
                return ab

            def emit_s2_pair(ch, abig, pi):
                """Row conv + mix for row tiles (2pi, 2pi+1), natural
                orientation.  One 4-bank PSUM tile [A0 A1 B0 B1]; each
                half opened by a full-width matmul (q2=0 lead filter,
                zero-padded rhs, start=True), compact band matmuls
                accumulate the rest.  Pair-wide mix + one 256-row DMA."""
                Z = ps.tile([128, NCHUNK * 512], f32, tag="ps",
                            name=f"z_{ch}_{pi}")
                for mi in range(2):
                    mtc = 2 * pi + mi
                    for pl, rs in enumerate(_PLANE_RS):
                        off = pl * 1024 + mi * 512
                        mms = [(q2, r) for q2 in range(NCHUNK) for r in rs]
                        last = mms[-1]
                        for q2, r in mms:
                            lhsT = abig[q2][:, r * H + 128 * mtc:
                                            r * H + 128 * mtc + 128]
                            if q2 == 0 and r == rs[0]:
                                oi = _OPENER_RS.index(r)
                                rhs = t2open[:, oi * H:(oi + 1) * H]
                                nc.tensor.matmul(
                                    Z[:, off:off + 512], lhsT, rhs,
                                    start=True, stop=((q2, r) == last),
                                    skip_group_check=True)
                            else:
                                rhs = t2c[:, q2 * R * BW + r * BW:
                                          q2 * R * BW + (r + 1) * BW]
                                c0 = _C0_S2[q2]
                                nc.tensor.matmul(
                                    Z[:, off + c0:off + c0 + BW], lhsT,
                                    rhs, start=False,
                                    stop=((q2, r) == last),
                                    skip_group_check=True)
                csl = coc[:, 1024 * pi:1024 * (pi + 1)]
                m = mpool.tile([128, 1024], f32, tag="m")
                nc.vector.tensor_tensor(m[:], Z[:, 1024:2048], csl,
                                        ALU.mult)
                acc = accpool.tile([128, 1024], bf16, tag="acc")
                nc.vector.tensor_tensor(acc[:], Z[:, 0:1024], m[:],
                                        ALU.add)
                nc.gpsimd.dma_start(
                    OUT[ch][256 * pi:256 * (pi + 1), :].rearrange(
                        "(m p) j -> p m j", p=128),
                    acc[:].rearrange("p (m j) -> p m j", m=2))

            # schedule: ch0 stage-1 prologue (drains alternate DVE/ACT
            # so the 2-deep PSUM pool turns over fast), then stage-2
            # pairs of channel k interleave with stage-1 of channel k+1
            ab0 = [emit_stage1(0, mt, drain_dve=(mt % 2 == 0))
                   for mt in range(NCHUNK)]
            ab1 = []
            ab1.append(emit_stage1(1, 0, False))
            ab1.append(emit_stage1(1, 1, False))
            emit_s2_pair(0, ab0, 0)
            ab1.append(emit_stage1(1, 2, False))
            ab1.append(emit_stage1(1, 3, False))
            emit_s2_pair(0, ab0, 1)
            ab2 = []
            ab2.append(emit_stage1(2, 0, False))
            ab2.append(emit_stage1(2, 1, False))
            emit_s2_pair(1, ab1, 0)
            ab2.append(emit_stage1(2, 2, False))
            ab2.append(emit_stage1(2, 3, False))
            emit_s2_pair(1, ab1, 1)
            emit_s2_pair(2, ab2, 0)
            emit_s2_pair(2, ab2, 1)

    nc.compile()
    return nc


_PROG = None


def _get_prog():
    global _PROG
    if _PROG is None:
        _PROG = _build()
    return _PROG


_TABLES = {}


def _get_tables(ws, bs):
    key = (float(ws), float(bs))
    if key not in _TABLES:
        t1, t2 = _filters(*key)
        _TABLES[key] = (_band_tables(t1, _C0_S1),
                        _band_tables(t2, _C0_S2),
                        _opener_table(t2))
    return _TABLES[key]


def make_in_maps(image, coc_map, w_sigma, b_sigma):
    bf = ml_dtypes.bfloat16
    tab1, tab2, t2open = _get_tables(
        float(np.asarray(w_sigma).reshape(-1)[0]),
        float(np.asarray(b_sigma).reshape(-1)[0]))
    image = np.asarray(image)
    coc_map = np.asarray(coc_map)
    in_maps = []
    for b in range(image.shape[0]):
        in_maps.append({
            "image": np.ascontiguousarray(image[b].astype(bf)),
            "coc": np.ascontiguousarray(coc_map[b, 0].astype(bf)),
            "t1c": tab1,
            "t2c": tab2,
            "t2open": t2open,
        })
    return in_maps


def kernel(image, coc_map, psf_params, w_sigma, b_sigma):
    from concourse.bass_utils import run_bass_kernel_spmd

    B = image.shape[0]
    assert image.shape == (8, 3, H, H)
    nc = _get_prog()
    in_maps = make_in_maps(image, coc_map, w_sigma, b_sigma)
    res = run_bass_kernel_spmd(nc, in_maps, core_ids=list(range(B)))
    out = np.stack([res.results[b]["out"] for b in range(B)], axis=0)
    return np.ascontiguousarray(out).astype(np.float32)


if __name__ == "__main__":
    _get_prog()
    print("build ok")


# revision 17
# speedup vs baseline: 1.1634x; 1.1465x over previous
"""FFT spatially-variant blur via a rank-3 linear-in-coc factorization.

Reference math: out = sum_k wbar_k(coc) * (psf_k (*) x), with mixture
weights wbar_k over 8 Gaussian PSF bases, sigma = clip(softplus(
0.3*coc + 0.5), 0.2, 12).  With coc in [0,1), sigma lies in
[0.974, 1.172]: the per-pixel effective kernel field is fit as

    K(c) ~= P0 + c * P1,   P0 rank-2, P1 rank-1  (field rel ~6.5e-3)

so the module becomes THREE separable convolutions (r=0,1 -> plane A;
r=2 -> plane B) plus a fused per-pixel mix  out = A + coc .* B.

Device schedule (per core = one batch sample, 3 channels):

  stage 1 (column conv, image stationary): per (ch, col-tile mt) one
    4-bank PSUM tile; bank q holds band cols of row-chunk q with the
    128-aligned layout C0[q] = 128q-16, so bank q's cols [16,144)
    are exactly output rows [128q, 128q+128).  Ten matmuls: four
    N=384 mains plus six N=45 seam matmuls that accumulate the
    cross-chunk band overlap straight into the neighbor bank, so the
    drain is ONE 4D copy [128, 3r, 4q, 128] -> ab bf16 per block.

  stage 2 (row conv, ab stationary, natural orientation): per
    (ch, mtc-PAIR) one 4-bank PSUM tile [A0 A1 B0 B1].  Each half is
    opened by a full-width N=512 matmul (zero-padded q2=0 band row,
    start=True resets the bank) and compact N=160 band matmuls
    accumulate the remaining (q2, r) contributions -- overlapping dst
    regions accumulate in PSUM.  The mix then runs at pair width:
    m = Bpair .* coc (DVE), acc = Apair + m (DVE, bf16), one 256-row
    output DMA.  Out partition = image row, so coc and the output
    stay in natural orientation (no host transposes).

Measured end-to-end rel err ~7e-3 vs the 2e-2 gate (bf16 + fit).

Data parallel: core b handles batch sample b (3 channels each).
"""

import numpy as np
import ml_dtypes

PSF_SIZE = 31
SIGMA_MIN = 0.2
SIGMA_MAX = 12.0
EPS = 1e-9
H = 512
NCHUNK = 4   # 512 / 128
R = 3        # separable filters: 0,1 -> plane A; 2 -> plane B
_PLANE_RS = [(0, 1), (2,)]
_OPENER_RS = [0, 2]   # plane-lead filters: opener rows of t2open
BW = 160

# stage-1 band layout: bank q covers output rows [128q-16, 128q+144)
_C0_S1 = [128 * q - 16 for q in range(NCHUNK)]
# stage-2 band layout: compact dst regions inside the 512-wide bank
_C0_S2 = [0, 113, 241, 352]


def _filters(ws, bs):
    """Rank-(2+1) linear-in-c factorization of the kernel field via
    alternating least squares: K(c) ~= P0 + c*P1 with P0 rank-2 and
    P1 rank-1.

    Returns (t1_taps[3][31], t2_taps[3][31]) fp64; filter r contributes
    outer(t1[r], t2[r]) to plane A (r<2) or plane B (r=2)."""
    lo = (-PSF_SIZE) // 2
    hi = PSF_SIZE // 2
    x = np.linspace(lo, hi, PSF_SIZE, dtype=np.float32).astype(np.float64)
    gx, gy = np.meshgrid(x, x, indexing='ij')
    sigmas = np.linspace(SIGMA_MIN, SIGMA_MAX, 8, dtype=np.float32)
    sigmas = sigmas.astype(np.float64)
    psfs = []
    for s in sigmas:
        g = np.exp(-(gx ** 2 + gy ** 2) / (2.0 * s * s + EPS))
        psfs.append(g / (g.sum() + EPS))
    psfs = np.array(psfs).reshape(8, -1)

    cg = np.linspace(0.0, 1.0, 2001)
    sig = np.clip(np.logaddexp(0.0, ws * cg + bs), SIGMA_MIN, SIGMA_MAX)
    w = np.exp(-(sig[:, None] - sigmas[None, :]) ** 2 / 2.0)
    w = w / (w.sum(1, keepdims=True) + EPS)
    M = w @ psfs                                     # [nc, 961]
    V = np.vander(cg, 2, increasing=True)            # [nc, 2]

    def proj(P, rank):
        evals, evecs = np.linalg.eigh(P.reshape(PSF_SIZE, PSF_SIZE))
        idx = np.argsort(-np.abs(evals))[:rank]
        flat = sum(evals[i] * np.outer(evecs[:, i], evecs[:, i])
                   for i in idx).reshape(-1)
        return flat, [(evals[i], evecs[:, i]) for i in idx]

    coef, *_ = np.linalg.lstsq(V, M, rcond=None)
    P0, P1 = coef[0], coef[1]
    for _ in range(200):
        P0r, _f = proj(P0, 2)
        P1f, *_ = np.linalg.lstsq(V[:, 1:2], M - V[:, 0:1] @ P0r[None, :],
                                  rcond=None)
        P1r, _f = proj(P1f[0], 1)
        P0f, *_ = np.linalg.lstsq(V[:, 0:1], M - V[:, 1:2] @ P1r[None, :],
                                  rcond=None)
        P0, P1 = P0f[0], P1f[0]
    _, f0 = proj(P0, 2)
    _, f1 = proj(P1, 1)
    t1, t2 = [], []
    for lam, u in f0 + f1:
        t1.append(u)
        t2.append(lam * u)
    return t1, t2


def _band_tables(taps_list, c0s):
    """Compact band tables [4 (q), 128, R*160] bf16:
    tab[q][p, r*160 + (c - c0s[q])] = taps_r[15 + c - (128q+p)]."""
    tab = np.zeros((NCHUNK, 128, R * BW), dtype=np.float64)
    for r, taps in enumerate(taps_list):
        for q in range(NCHUNK):
            c0 = c0s[q]
            for p in range(128):
                row = 128 * q + p
                j0 = max(c0, row - 15, 0)
                j1 = min(c0 + BW, row + 16, H)
                if j1 > j0:
                    tab[q, p, r * BW + j0 - c0:
                        r * BW + j1 - c0] = \
                        taps[15 + np.arange(j0, j1) - row]
    return tab.astype(ml_dtypes.bfloat16)


def _opener_table(taps_list):
    """Full-width zero-padded q=0 band rows for the plane-opening
    matmuls: open[p, i, c'] = taps_{OPENER_RS[i]}[15 + c' - p]."""
    open_ = np.zeros((128, len(_OPENER_RS), H), dtype=np.float64)
    for i, r in enumerate(_OPENER_RS):
        taps = taps_list[r]
        for p in range(128):
            j0 = max(0, p - 15)
            j1 = min(H, p + 16)
            open_[p, i, j0:j1] = taps[15 + np.arange(j0, j1) - p]
    return open_.astype(ml_dtypes.bfloat16)


def _build():
    import concourse.bass as bass  # noqa: F401
    import concourse.tile as tile
    from concourse import mybir, bacc

    f32 = mybir.dt.float32
    bf16 = mybir.dt.bfloat16
    AF = mybir.ActivationFunctionType
    ALU = mybir.AluOpType

    nc = bacc.Bacc("TRN2", target_bir_lowering=False, debug=False,
                   disable_frame_to_traceback=True)
    IMG = nc.declare_dram_parameter("image", [3, H, H], bf16, isOutput=False)
    COC = nc.declare_dram_parameter("coc", [H, H], bf16, isOutput=False)
    T1C = nc.declare_dram_parameter("t1c", [NCHUNK, 128, R * BW], bf16,
                                    isOutput=False)
    T2C = nc.declare_dram_parameter("t2c", [NCHUNK, 128, R * BW], bf16,
                                    isOutput=False)
    T2O = nc.declare_dram_parameter("t2open", [128, len(_OPENER_RS), H],
                                    bf16, isOutput=False)
    OUT = nc.declare_dram_parameter("out", [3, H, H], bf16, isOutput=True)

    with tile.TileContext(nc) as tc:
        import contextlib
        ctx = contextlib.ExitStack()
        with ctx:
            tpool = ctx.enter_context(tc.tile_pool(name="ttab", bufs=1))
            cpool = ctx.enter_context(tc.tile_pool(name="coc", bufs=1))
            xpool = ctx.enter_context(tc.tile_pool(name="xin", bufs=1))
            apool = ctx.enter_context(tc.tile_pool(name="abig", bufs=12))
            mpool = ctx.enter_context(tc.tile_pool(name="mtmp", bufs=2))
            accpool = ctx.enter_context(tc.tile_pool(name="acc", bufs=2))
            # 4 two-bank PSUM units keep stage-1 halves and stage-2
            # pairs pipelining without PE stalls on drain latency
            ps = ctx.enter_context(
                tc.tile_pool(name="ps", bufs=4, space="PSUM"))

            t1c = tpool.tile([128, NCHUNK * R * BW], bf16, tag="t1c")
            t2c = tpool.tile([128, NCHUNK * R * BW], bf16, tag="t2c")
            t2open = tpool.tile([128, len(_OPENER_RS) * H], bf16, tag="t2o")
            xs = [xpool.tile([128, NCHUNK * H], bf16, tag=f"xs{ch}",
                             name=f"xs{ch}")
                  for ch in range(3)]
            coc = cpool.tile([128, NCHUNK * H], bf16, tag="coc")

            # --- input DMAs.  The first stage-1 matmul gates on only
            # t1c[q0] + the first image column block; queue assignment
            # keeps the scalar queue free for drains after its 5 issues.
            for q in range(NCHUNK):
                nc.sync.dma_start(t1c[:, q * R * BW:(q + 1) * R * BW],
                                  T1C[q])
            nc.scalar.dma_start(xs[0][:, 0:128], IMG[0][0:128, 0:128])
            nc.scalar.dma_start(xs[0][:, 128:H], IMG[0][0:128, 128:])
            for q in range(1, NCHUNK):
                nc.scalar.dma_start(xs[0][:, q * H:(q + 1) * H],
                                    IMG[0][128 * q:128 * (q + 1), :])
            for q in range(NCHUNK):
                nc.sync.dma_start(xs[1][:, q * H:(q + 1) * H],
                                  IMG[1][128 * q:128 * (q + 1), :])
            for q in range(NCHUNK):
                nc.sync.dma_start(xs[2][:, q * H:(q + 1) * H],
                                  IMG[2][128 * q:128 * (q + 1), :])
            nc.gpsimd.dma_start(
                t2c[:].rearrange("p (q j) -> p q j", q=NCHUNK),
                T2C.rearrange("q p j -> p q j"))
            nc.gpsimd.dma_start(t2open[:],
                                T2O.rearrange("p i j -> p (i j)"))
            nc.gpsimd.dma_start(
                coc[:].rearrange("p (q j) -> p q j", q=NCHUNK),
                COC.rearrange("(q p) j -> p q j", p=128))

            abs_ = {}

            def emit_s1_half(ch, mt, hf, drain_dve):
                """Column-conv half block: banks (2hf, 2hf+1) of the
                128-aligned band layout on one 2-bank unit.  Chunk q's
                main writes bank q cols [16,144); cross-chunk seams
                accumulate into the neighbor bank (N=45); one 4D drain
                copies rows c = [256hf, 256hf+256) into ab."""
                P = ps.tile([128, 1024], f32, tag="ps",
                            name=f"b1_{ch}_{mt}_{hf}")
                qs = (2 * hf, 2 * hf + 1)

                def pview(q, j0, j1):
                    off = (q - qs[0]) * 512
                    return P[:, off:off + R * BW].rearrange(
                        "p (r j) -> p r j", r=R)[:, :, j0:j1]

                def tview(q, j0, j1):
                    return t1c[:, q * R * BW:(q + 1) * R * BW].rearrange(
                        "p (r j) -> p r j", r=R)[:, :, j0:j1]

                def lhs(q):
                    return xs[ch][:, q * H + 128 * mt:
                                  q * H + 128 * mt + 128]

                # mains first (start=True resets [16,144)), then seams
                for q in qs:
                    nc.tensor.matmul(pview(q, 16, 144), lhs(q),
                                     tview(q, 16, 144),
                                     start=True, stop=False,
                                     skip_group_check=True)
                # seams (chunk s -> bank b): s=b+1 lands in bank cols
                # [129,144) from its table cols [1,16); s=b-1 lands in
                # [16,31) from cols [144,159)
                if hf == 0:
                    seams = [(0, 1, False), (1, 0, True), (2, 1, True)]
                else:
                    seams = [(1, 2, False), (3, 2, True), (2, 3, True)]
                for s, b, stop in seams:
                    j0, sj0 = (129, 1) if s > b else (16, 144)
                    nc.tensor.matmul(pview(b, j0, j0 + 15), lhs(s),
                                     tview(s, sj0, sj0 + 15),
                                     start=False, stop=stop,
                                     skip_group_check=True)
                if (ch, mt) not in abs_:
                    abs_[(ch, mt)] = apool.tile([128, R * H], bf16,
                                                tag="ab",
                                                name=f"ab{ch}_{mt}")
                ab = abs_[(ch, mt)]
                src = P[:].rearrange("p (q x) -> p q x", q=2)[
                    :, :, 0:R * BW].rearrange(
                    "p q (r j) -> p r q j", r=R)[:, :, :, 16:144]
                dst = ab[:].rearrange("p (r qq j) -> p r qq j",
                                      qq=NCHUNK, j=128)[:, :, qs[0]:qs[1] + 1]
                if drain_dve:
                    nc.vector.tensor_copy(dst, src)
                else:
                    nc.scalar.activation(dst, src, AF.Copy)
                return ab

            def emit_s2_mm(ch, pi, pl, Z):
                """Row-conv matmuls for plane pl of row tiles
                (2pi, 2pi+1) into the 2-bank unit Z (cols [mi*512..])."""
                abig = [abs_[(ch, mt)] for mt in range(NCHUNK)]
                rs = _PLANE_RS[pl]
                for mi in range(2):
                    mtc = 2 * pi + mi
                    off = mi * 512
                    mms = [(q2, r) for q2 in range(NCHUNK) for r in rs]
                    last = mms[-1]
                    for q2, r in mms:
                        lhsT = abig[q2][:, r * H + 128 * mtc:
                                        r * H + 128 * mtc + 128]
                        if q2 == 0 and r == rs[0]:
                            oi = _OPENER_RS.index(r)
                            rhs = t2open[:, oi * H:(oi + 1) * H]
                            nc.tensor.matmul(
                                Z[:, off:off + 512], lhsT, rhs,
                                start=True, stop=((q2, r) == last),
                                skip_group_check=True)
                        else:
                            rhs = t2c[:, q2 * R * BW + r * BW:
                                      q2 * R * BW + (r + 1) * BW]
                            c0 = _C0_S2[q2]
                            nc.tensor.matmul(
                                Z[:, off + c0:off + c0 + BW], lhsT,
                                rhs, start=False,
                                stop=((q2, r) == last),
                                skip_group_check=True)

            s2state = {}

            def emit_s2_B(ch, pi):
                """Plane B for a pair on a fresh 2-bank unit, then the
                pair-wide multiply m = B .* coc frees the unit for A."""
                Z = ps.tile([128, 1024], f32, tag="ps",
                            name=f"z_{ch}_{pi}")
                emit_s2_mm(ch, pi, 1, Z)
                csl = coc[:, 1024 * pi:1024 * (pi + 1)]
                m = mpool.tile([128, 1024], f32, tag="m")
                nc.vector.tensor_tensor(m[:], Z[:], csl, ALU.mult)
                s2state[(ch, pi)] = (Z, m)

            def emit_s2_A(ch, pi):
                """Plane A reuses the pair's unit (openers reset it),
                then acc = A + m and one 256-row output DMA."""
                Z, m = s2state.pop((ch, pi))
                emit_s2_mm(ch, pi, 0, Z)
                acc = accpool.tile([128, 1024], bf16, tag="acc")
                nc.vector.tensor_tensor(acc[:], Z[:], m[:], ALU.add)
                nc.gpsimd.dma_start(
                    OUT[ch][256 * pi:256 * (pi + 1), :].rearrange(
                        "(m p) j -> p m j", p=128),
                    acc[:].rearrange("p (m j) -> p m j", m=2))

            # schedule: ch0 stage-1 prologue with drains alternating
            # ACT/DVE for fast unit turnover; then each stage-2 pair's
            # B and A matmul groups are separated by four stage-1
            # halves of the next channel so the PE never waits on the
            # m = B.*coc multiply that frees the pair's PSUM unit.
            def h(ch, mt, hf, dve=False):
                emit_s1_half(ch, mt, hf, dve)

            for mt in range(NCHUNK):
                h(0, mt, 0, dve=False)
                h(0, mt, 1, dve=True)
            emit_s2_B(0, 0)
            h(1, 0, 0); h(1, 0, 1, dve=True); h(1, 1, 0); h(1, 1, 1)
            emit_s2_A(0, 0)
            emit_s2_B(0, 1)
            h(1, 2, 0); h(1, 2, 1, dve=True); h(1, 3, 0); h(1, 3, 1)
            emit_s2_A(0, 1)
            emit_s2_B(1, 0)
            h(2, 0, 0); h(2, 0, 1, dve=True); h(2, 1, 0); h(2, 1, 1)
            emit_s2_A(1, 0)
            emit_s2_B(1, 1)
            h(2, 2, 0); h(2, 2, 1, dve=True); h(2, 3, 0); h(2, 3, 1)
            emit_s2_A(1, 1)
            emit_s2_B(2, 0)
            emit_s2_B(2, 1)
            emit_s2_A(2, 0)
            emit_s2_A(2, 1)

    nc.compile()
    return nc


_PROG = None


def _get_prog():
    global _PROG
    if _PROG is None:
        _PROG = _build()
    return _PROG


_TABLES = {}


def _get_tables(ws, bs):
    key = (float(ws), float(bs))
    if key not in _TABLES:
        t1, t2 = _filters(*key)
        _TABLES[key] = (_band_tables(t1, _C0_S1),
                        _band_tables(t2, _C0_S2),
                        _opener_table(t2))
    return _TABLES[key]


def make_in_maps(image, coc_map, w_sigma, b_sigma):
    bf = ml_dtypes.bfloat16
    tab1, tab2, t2open = _get_tables(
        float(np.asarray(w_sigma).reshape(-1)[0]),
        float(np.asarray(b_sigma).reshape(-1)[0]))
    image = np.asarray(image)
    coc_map = np.asarray(coc_map)
    in_maps = []
    for b in range(image.shape[0]):
        in_maps.append({
            "image": np.ascontiguousarray(image[b].astype(bf)),
            "coc": np.ascontiguousarray(coc_map[b, 0].astype(bf)),
            "t1c": tab1,
            "t2c": tab2,
            "t2open": t2open,
        })
    return in_maps


def kernel(image, coc_map, psf_params, w_sigma, b_sigma):
    from concourse.bass_utils import run_bass_kernel_spmd

    B = image.shape[0]
    assert image.shape == (8, 3, H, H)
    nc = _get_prog()
    in_maps = make_in_maps(image, coc_map, w_sigma, b_sigma)
    res = run_bass_kernel_spmd(nc, in_maps, core_ids=list(range(B)))
    out = np.stack([res.results[b]["out"] for b in range(B)], axis=0)
    return np.ascontiguousarray(out).astype(np.float32)


if __name__ == "__main__":
    _get_prog()
    print("build ok")


# revision 18
# speedup vs baseline: 1.2513x; 1.0756x over previous
"""FFT spatially-variant blur via a rank-3 linear-in-coc factorization.

Reference math: out = sum_k wbar_k(coc) * (psf_k (*) x), with mixture
weights wbar_k over 8 Gaussian PSF bases, sigma = clip(softplus(
0.3*coc + 0.5), 0.2, 12).  With coc in [0,1), sigma lies in
[0.974, 1.172]: the per-pixel effective kernel field is fit as

    K(c) ~= P0 + c * P1,   P0 rank-2, P1 rank-1  (field rel ~6.5e-3)

so the module becomes THREE separable convolutions (r=0,1 -> plane A;
r=2 -> plane B) plus a fused per-pixel mix  out = A + coc .* B.

Device schedule (per core = one batch sample, 3 channels):

  stage 1 (column conv, image stationary): per (ch, col-tile mt) one
    4-bank PSUM tile; bank q holds band cols of row-chunk q with the
    128-aligned layout C0[q] = 128q-16, so bank q's cols [16,144)
    are exactly output rows [128q, 128q+128).  Ten matmuls: four
    N=384 mains plus six N=45 seam matmuls that accumulate the
    cross-chunk band overlap straight into the neighbor bank, so the
    drain is ONE 4D copy [128, 3r, 4q, 128] -> ab bf16 per block.

  stage 2 (row conv, ab stationary, natural orientation): per
    (ch, mtc-PAIR) one 4-bank PSUM tile [A0 A1 B0 B1].  Each half is
    opened by a full-width N=512 matmul (zero-padded q2=0 band row,
    start=True resets the bank) and compact N=160 band matmuls
    accumulate the remaining (q2, r) contributions -- overlapping dst
    regions accumulate in PSUM.  The mix then runs at pair width:
    m = Bpair .* coc (DVE), acc = Apair + m (DVE, bf16), one 256-row
    output DMA.  Out partition = image row, so coc and the output
    stay in natural orientation (no host transposes).

Measured end-to-end rel err ~7e-3 vs the 2e-2 gate (bf16 + fit).

Data parallel: core b handles batch sample b (3 channels each).
"""

import numpy as np
import ml_dtypes

PSF_SIZE = 31
SIGMA_MIN = 0.2
SIGMA_MAX = 12.0
EPS = 1e-9
H = 512
NCHUNK = 4   # 512 / 128
R = 3        # separable filters: 0,1 -> plane A; 2 -> plane B
_PLANE_RS = [(0, 1), (2,)]
_OPENER_RS = [0, 2]   # plane-lead filters: opener rows of t2open
BW = 160

# stage-1 band layout: bank q covers output rows [128q-16, 128q+144)
_C0_S1 = [128 * q - 16 for q in range(NCHUNK)]
# stage-2 band layout: compact dst regions inside the 512-wide bank
_C0_S2 = [0, 113, 241, 352]


def _filters(ws, bs):
    """Rank-(2+1) linear-in-c factorization of the kernel field via
    alternating least squares: K(c) ~= P0 + c*P1 with P0 rank-2 and
    P1 rank-1.

    Returns (t1_taps[3][31], t2_taps[3][31]) fp64; filter r contributes
    outer(t1[r], t2[r]) to plane A (r<2) or plane B (r=2)."""
    lo = (-PSF_SIZE) // 2
    hi = PSF_SIZE // 2
    x = np.linspace(lo, hi, PSF_SIZE, dtype=np.float32).astype(np.float64)
    gx, gy = np.meshgrid(x, x, indexing='ij')
    sigmas = np.linspace(SIGMA_MIN, SIGMA_MAX, 8, dtype=np.float32)
    sigmas = sigmas.astype(np.float64)
    psfs = []
    for s in sigmas:
        g = np.exp(-(gx ** 2 + gy ** 2) / (2.0 * s * s + EPS))
        psfs.append(g / (g.sum() + EPS))
    psfs = np.array(psfs).reshape(8, -1)

    cg = np.linspace(0.0, 1.0, 2001)
    sig = np.clip(np.logaddexp(0.0, ws * cg + bs), SIGMA_MIN, SIGMA_MAX)
    w = np.exp(-(sig[:, None] - sigmas[None, :]) ** 2 / 2.0)
    w = w / (w.sum(1, keepdims=True) + EPS)
    M = w @ psfs                                     # [nc, 961]
    V = np.vander(cg, 2, increasing=True)            # [nc, 2]

    def proj(P, rank):
        evals, evecs = np.linalg.eigh(P.reshape(PSF_SIZE, PSF_SIZE))
        idx = np.argsort(-np.abs(evals))[:rank]
        flat = sum(evals[i] * np.outer(evecs[:, i], evecs[:, i])
                   for i in idx).reshape(-1)
        return flat, [(evals[i], evecs[:, i]) for i in idx]

    coef, *_ = np.linalg.lstsq(V, M, rcond=None)
    P0, P1 = coef[0], coef[1]
    for _ in range(200):
        P0r, _f = proj(P0, 2)
        P1f, *_ = np.linalg.lstsq(V[:, 1:2], M - V[:, 0:1] @ P0r[None, :],
                                  rcond=None)
        P1r, _f = proj(P1f[0], 1)
        P0f, *_ = np.linalg.lstsq(V[:, 0:1], M - V[:, 1:2] @ P1r[None, :],
                                  rcond=None)
        P0, P1 = P0f[0], P1f[0]
    _, f0 = proj(P0, 2)
    _, f1 = proj(P1, 1)
    t1, t2 = [], []
    for lam, u in f0 + f1:
        t1.append(u)
        t2.append(lam * u)
    return t1, t2


def _band_tables(taps_list, c0s):
    """Compact band tables [4 (q), 128, R*160] bf16:
    tab[q][p, r*160 + (c - c0s[q])] = taps_r[15 + c - (128q+p)]."""
    tab = np.zeros((NCHUNK, 128, R * BW), dtype=np.float64)
    for r, taps in enumerate(taps_list):
        for q in range(NCHUNK):
            c0 = c0s[q]
            for p in range(128):
                row = 128 * q + p
                j0 = max(c0, row - 15, 0)
                j1 = min(c0 + BW, row + 16, H)
                if j1 > j0:
                    tab[q, p, r * BW + j0 - c0:
                        r * BW + j1 - c0] = \
                        taps[15 + np.arange(j0, j1) - row]
    return tab.astype(ml_dtypes.bfloat16)


def _opener_table(taps_list):
    """Full-width zero-padded q=0 band rows for the plane-opening
    matmuls: open[p, i, c'] = taps_{OPENER_RS[i]}[15 + c' - p]."""
    open_ = np.zeros((128, len(_OPENER_RS), H), dtype=np.float64)
    for i, r in enumerate(_OPENER_RS):
        taps = taps_list[r]
        for p in range(128):
            j0 = max(0, p - 15)
            j1 = min(H, p + 16)
            open_[p, i, j0:j1] = taps[15 + np.arange(j0, j1) - p]
    return open_.astype(ml_dtypes.bfloat16)


def _build():
    import concourse.bass as bass  # noqa: F401
    import concourse.tile as tile
    from concourse import mybir, bacc

    f32 = mybir.dt.float32
    bf16 = mybir.dt.bfloat16
    AF = mybir.ActivationFunctionType
    ALU = mybir.AluOpType

    nc = bacc.Bacc("TRN2", target_bir_lowering=False, debug=False,
                   disable_frame_to_traceback=True)
    IMG = nc.declare_dram_parameter("image", [3, H, H], bf16, isOutput=False)
    COC = nc.declare_dram_parameter("coc", [H, H], bf16, isOutput=False)
    T1C = nc.declare_dram_parameter("t1c", [NCHUNK, 128, R * BW], bf16,
                                    isOutput=False)
    T2C = nc.declare_dram_parameter("t2c", [NCHUNK, 128, R * BW], bf16,
                                    isOutput=False)
    T2O = nc.declare_dram_parameter("t2open", [128, len(_OPENER_RS), H],
                                    bf16, isOutput=False)
    OUT = nc.declare_dram_parameter("out", [3, H, H], bf16, isOutput=True)

    with tile.TileContext(nc) as tc:
        import contextlib
        ctx = contextlib.ExitStack()
        with ctx:
            tpool = ctx.enter_context(tc.tile_pool(name="ttab", bufs=1))
            cpool = ctx.enter_context(tc.tile_pool(name="coc", bufs=1))
            xpool = ctx.enter_context(tc.tile_pool(name="xin", bufs=1))
            apool = ctx.enter_context(tc.tile_pool(name="abig", bufs=12))
            mpool = ctx.enter_context(tc.tile_pool(name="mtmp", bufs=2))
            accpool = ctx.enter_context(tc.tile_pool(name="acc", bufs=2))
            # 4 two-bank PSUM units keep stage-1 halves and stage-2
            # pairs pipelining without PE stalls on drain latency
            ps = ctx.enter_context(
                tc.tile_pool(name="ps", bufs=4, space="PSUM"))

            t1c = tpool.tile([128, NCHUNK * R * BW], bf16, tag="t1c")
            t2c = tpool.tile([128, NCHUNK * R * BW], bf16, tag="t2c")
            t2open = tpool.tile([128, len(_OPENER_RS) * H], bf16, tag="t2o")
            xs = [xpool.tile([128, NCHUNK * H], bf16, tag=f"xs{ch}",
                             name=f"xs{ch}")
                  for ch in range(3)]
            coc = cpool.tile([128, NCHUNK * H], bf16, tag="coc")

            # --- input DMAs: one large transfer per tensor (issue cost
            # ~0.8us each dominates; per-issue latency starved the
            # prologue when loads were chunked).
            def img_load(engine, ch):
                engine.dma_start(
                    xs[ch][:].rearrange("p (q j) -> p q j", q=NCHUNK),
                    IMG[ch].rearrange("(q p) j -> p q j", p=128))

            nc.sync.dma_start(
                t1c[:].rearrange("p (q j) -> p q j", q=NCHUNK),
                T1C.rearrange("q p j -> p q j"))
            img_load(nc.scalar, 0)
            img_load(nc.sync, 1)
            nc.sync.dma_start(
                t2c[:].rearrange("p (q j) -> p q j", q=NCHUNK),
                T2C.rearrange("q p j -> p q j"))
            nc.gpsimd.dma_start(t2open[:],
                                T2O.rearrange("p i j -> p (i j)"))
            nc.gpsimd.dma_start(
                coc[:].rearrange("p (q j) -> p q j", q=NCHUNK),
                COC.rearrange("(q p) j -> p q j", p=128))
            img_load(nc.gpsimd, 2)

            abs_ = {}

            def emit_s1_half(ch, mt, hf, drain_dve):
                """Column-conv half block: banks (2hf, 2hf+1) of the
                128-aligned band layout on one 2-bank unit.  Chunk q's
                main writes bank q cols [16,144); cross-chunk seams
                accumulate into the neighbor bank (N=45); one 4D drain
                copies rows c = [256hf, 256hf+256) into ab."""
                P = ps.tile([128, 1024], f32, tag="ps",
                            name=f"b1_{ch}_{mt}_{hf}")
                qs = (2 * hf, 2 * hf + 1)

                def pview(q, j0, j1):
                    off = (q - qs[0]) * 512
                    return P[:, off:off + R * BW].rearrange(
                        "p (r j) -> p r j", r=R)[:, :, j0:j1]

                def tview(q, j0, j1):
                    return t1c[:, q * R * BW:(q + 1) * R * BW].rearrange(
                        "p (r j) -> p r j", r=R)[:, :, j0:j1]

                def lhs(q):
                    return xs[ch][:, q * H + 128 * mt:
                                  q * H + 128 * mt + 128]

                # mains first (start=True resets [16,144)), then seams
                for q in qs:
                    nc.tensor.matmul(pview(q, 16, 144), lhs(q),
                                     tview(q, 16, 144),
                                     start=True, stop=False,
                                     skip_group_check=True)
                # seams (chunk s -> bank b): s=b+1 lands in bank cols
                # [129,144) from its table cols [1,16); s=b-1 lands in
                # [16,31) from cols [144,159)
                if hf == 0:
                    seams = [(0, 1, False), (1, 0, True), (2, 1, True)]
                else:
                    seams = [(1, 2, False), (3, 2, True), (2, 3, True)]
                for s, b, stop in seams:
                    j0, sj0 = (129, 1) if s > b else (16, 144)
                    nc.tensor.matmul(pview(b, j0, j0 + 15), lhs(s),
                                     tview(s, sj0, sj0 + 15),
                                     start=False, stop=stop,
                                     skip_group_check=True)
                if (ch, mt) not in abs_:
                    abs_[(ch, mt)] = apool.tile([128, R * H], bf16,
                                                tag="ab",
                                                name=f"ab{ch}_{mt}")
                ab = abs_[(ch, mt)]
                src = P[:].rearrange("p (q x) -> p q x", q=2)[
                    :, :, 0:R * BW].rearrange(
                    "p q (r j) -> p r q j", r=R)[:, :, :, 16:144]
                dst = ab[:].rearrange("p (r qq j) -> p r qq j",
                                      qq=NCHUNK, j=128)[:, :, qs[0]:qs[1] + 1]
                if drain_dve:
                    nc.vector.tensor_copy(dst, src)
                else:
                    nc.scalar.activation(dst, src, AF.Copy)
                return ab

            def emit_s2_mm(ch, pi, pl, Z):
                """Row-conv matmuls for plane pl of row tiles
                (2pi, 2pi+1) into the 2-bank unit Z (cols [mi*512..])."""
                abig = [abs_[(ch, mt)] for mt in range(NCHUNK)]
                rs = _PLANE_RS[pl]
                for mi in range(2):
                    mtc = 2 * pi + mi
                    off = mi * 512
                    mms = [(q2, r) for q2 in range(NCHUNK) for r in rs]
                    last = mms[-1]
                    for q2, r in mms:
                        lhsT = abig[q2][:, r * H + 128 * mtc:
                                        r * H + 128 * mtc + 128]
                        if q2 == 0 and r == rs[0]:
                            oi = _OPENER_RS.index(r)
                            rhs = t2open[:, oi * H:(oi + 1) * H]
                            nc.tensor.matmul(
                                Z[:, off:off + 512], lhsT, rhs,
                                start=True, stop=((q2, r) == last),
                                skip_group_check=True)
                        else:
                            rhs = t2c[:, q2 * R * BW + r * BW:
                                      q2 * R * BW + (r + 1) * BW]
                            c0 = _C0_S2[q2]
                            nc.tensor.matmul(
                                Z[:, off + c0:off + c0 + BW], lhsT,
                                rhs, start=False,
                                stop=((q2, r) == last),
                                skip_group_check=True)

            s2state = {}

            def emit_s2_B(ch, pi):
                """Plane B for a pair on a fresh 2-bank unit, then the
                pair-wide multiply m = B .* coc frees the unit for A."""
                Z = ps.tile([128, 1024], f32, tag="ps",
                            name=f"z_{ch}_{pi}")
                emit_s2_mm(ch, pi, 1, Z)
                csl = coc[:, 1024 * pi:1024 * (pi + 1)]
                m = mpool.tile([128, 1024], f32, tag="m")
                nc.vector.tensor_tensor(m[:], Z[:], csl, ALU.mult)
                s2state[(ch, pi)] = (Z, m)

            def emit_s2_A(ch, pi):
                """Plane A reuses the pair's unit (openers reset it),
                then acc = A + m and one 256-row output DMA."""
                Z, m = s2state.pop((ch, pi))
                emit_s2_mm(ch, pi, 0, Z)
                acc = accpool.tile([128, 1024], bf16, tag="acc")
                nc.vector.tensor_tensor(acc[:], Z[:], m[:], ALU.add)
                nc.gpsimd.dma_start(
                    OUT[ch][256 * pi:256 * (pi + 1), :].rearrange(
                        "(m p) j -> p m j", p=128),
                    acc[:].rearrange("p (m j) -> p m j", m=2))

            # schedule: ch0 stage-1 prologue with drains alternating
            # ACT/DVE for fast unit turnover; then each stage-2 pair's
            # B and A matmul groups are separated by four stage-1
            # halves of the next channel so the PE never waits on the
            # m = B.*coc multiply that frees the pair's PSUM unit.
            def h(ch, mt, hf, dve=False):
                emit_s1_half(ch, mt, hf, dve)

            for mt in range(NCHUNK):
                h(0, mt, 0, dve=False)
                h(0, mt, 1, dve=True)
            emit_s2_B(0, 0)
            h(1, 0, 0); h(1, 0, 1, dve=True); h(1, 1, 0); h(1, 1, 1)
            emit_s2_A(0, 0)
            emit_s2_B(0, 1)
            h(1, 2, 0); h(1, 2, 1, dve=True); h(1, 3, 0); h(1, 3, 1)
            emit_s2_A(0, 1)
            emit_s2_B(1, 0)
            h(2, 0, 0); h(2, 0, 1, dve=True); h(2, 1, 0); h(2, 1, 1)
            emit_s2_A(1, 0)
            emit_s2_B(1, 1)
            h(2, 2, 0); h(2, 2, 1, dve=True); h(2, 3, 0); h(2, 3, 1)
            emit_s2_A(1, 1)
            emit_s2_B(2, 0)
            emit_s2_B(2, 1)
            emit_s2_A(2, 0)
            emit_s2_A(2, 1)

    nc.compile()
    return nc


_PROG = None


def _get_prog():
    global _PROG
    if _PROG is None:
        _PROG = _build()
    return _PROG


_TABLES = {}


def _get_tables(ws, bs):
    key = (float(ws), float(bs))
    if key not in _TABLES:
        t1, t2 = _filters(*key)
        _TABLES[key] = (_band_tables(t1, _C0_S1),
                        _band_tables(t2, _C0_S2),
                        _opener_table(t2))
    return _TABLES[key]


def make_in_maps(image, coc_map, w_sigma, b_sigma):
    bf = ml_dtypes.bfloat16
    tab1, tab2, t2open = _get_tables(
        float(np.asarray(w_sigma).reshape(-1)[0]),
        float(np.asarray(b_sigma).reshape(-1)[0]))
    image = np.asarray(image)
    coc_map = np.asarray(coc_map)
    in_maps = []
    for b in range(image.shape[0]):
        in_maps.append({
            "image": np.ascontiguousarray(image[b].astype(bf)),
            "coc": np.ascontiguousarray(coc_map[b, 0].astype(bf)),
            "t1c": tab1,
            "t2c": tab2,
            "t2open": t2open,
        })
    return in_maps


def kernel(image, coc_map, psf_params, w_sigma, b_sigma):
    from concourse.bass_utils import run_bass_kernel_spmd

    B = image.shape[0]
    assert image.shape == (8, 3, H, H)
    nc = _get_prog()
    in_maps = make_in_maps(image, coc_map, w_sigma, b_sigma)
    res = run_bass_kernel_spmd(nc, in_maps, core_ids=list(range(B)))
    out = np.stack([res.results[b]["out"] for b in range(B)], axis=0)
    return np.ascontiguousarray(out).astype(np.float32)


if __name__ == "__main__":
    _get_prog()
    print("build ok")


# revision 29
# speedup vs baseline: 1.2572x; 1.0047x over previous
"""FFT spatially-variant blur via a rank-3 linear-in-coc factorization.

Reference math: out = sum_k wbar_k(coc) * (psf_k (*) x), with mixture
weights wbar_k over 8 Gaussian PSF bases, sigma = clip(softplus(
0.3*coc + 0.5), 0.2, 12).  With coc in [0,1), sigma lies in
[0.974, 1.172]: the per-pixel effective kernel field is fit as

    K(c) ~= P0 + c * P1,   P0 rank-2, P1 rank-1  (field rel ~6.5e-3)

so the module becomes THREE separable convolutions (r=0,1 -> plane A;
r=2 -> plane B) plus a fused per-pixel mix  out = A + coc .* B.

Device schedule (per core = one batch sample, 3 channels):

  stage 1 (column conv, image stationary): per (ch, col-tile mt) one
    4-bank PSUM tile; bank q holds band cols of row-chunk q with the
    128-aligned layout C0[q] = 128q-16, so bank q's cols [16,144)
    are exactly output rows [128q, 128q+128).  Ten matmuls: four
    N=384 mains plus six N=45 seam matmuls that accumulate the
    cross-chunk band overlap straight into the neighbor bank, so the
    drain is ONE 4D copy [128, 3r, 4q, 128] -> ab bf16 per block.

  stage 2 (row conv, ab stationary, natural orientation): per
    (ch, mtc-PAIR) one 4-bank PSUM tile [A0 A1 B0 B1].  Each half is
    opened by a full-width N=512 matmul (zero-padded q2=0 band row,
    start=True resets the bank) and compact N=160 band matmuls
    accumulate the remaining (q2, r) contributions -- overlapping dst
    regions accumulate in PSUM.  The mix then runs at pair width:
    m = Bpair .* coc (DVE), acc = Apair + m (DVE, bf16), one 256-row
    output DMA.  Out partition = image row, so coc and the output
    stay in natural orientation (no host transposes).

Measured end-to-end rel err ~7e-3 vs the 2e-2 gate (bf16 + fit).

Data parallel: core b handles batch sample b (3 channels each).
"""

import numpy as np
import ml_dtypes

PSF_SIZE = 31
SIGMA_MIN = 0.2
SIGMA_MAX = 12.0
EPS = 1e-9
H = 512
NCHUNK = 4   # 512 / 128
R = 2        # separable filters: plane i = f_i(coc) * (u_i (*) x)
_PLANE_RS = [(0,), (1,)]
_OPENER_RS = [0, 1]   # plane-lead filters: opener rows of t2open
BW = 160

# stage-1 band layout: bank q covers output rows [128q-16, 128q+144)
_C0_S1 = [128 * q - 16 for q in range(NCHUNK)]
# stage-2 band layout: compact dst regions inside the 512-wide bank
_C0_S2 = [0, 113, 241, 352]


def _filters(ws, bs):
    """Two-term symmetric rank-1 factorization of the kernel field
    with LINEAR coefficient functions, fit by alternating least
    squares:  K(c) ~= f1(c) u1u1' + f2(c) u2u2',  f_i(c) = a_i c + b_i
    (field rel err ~3.1e-3, better than the rank-(2+1) linear-in-c
    model at 6.5e-3).

    Returns (taps[2][31] unit vectors, lin[2] = (a_i, b_i)); the
    separable conv uses u_i for both row and column taps, and the
    scale lives entirely in the per-pixel maps f_i(coc)."""
    lo = (-PSF_SIZE) // 2
    hi = PSF_SIZE // 2
    x = np.linspace(lo, hi, PSF_SIZE, dtype=np.float32).astype(np.float64)
    gx, gy = np.meshgrid(x, x, indexing='ij')
    sigmas = np.linspace(SIGMA_MIN, SIGMA_MAX, 8, dtype=np.float32)
    sigmas = sigmas.astype(np.float64)
    psfs = []
    for s in sigmas:
        g = np.exp(-(gx ** 2 + gy ** 2) / (2.0 * s * s + EPS))
        psfs.append(g / (g.sum() + EPS))
    psfs = np.array(psfs).reshape(8, -1)

    cg = np.linspace(0.0, 1.0, 201)
    sig = np.clip(np.logaddexp(0.0, ws * cg + bs), SIGMA_MIN, SIGMA_MAX)
    w = np.exp(-(sig[:, None] - sigmas[None, :]) ** 2 / 2.0)
    w = w / (w.sum(1, keepdims=True) + EPS)
    M = w @ psfs                                     # [nc, 961]

    U_, S_, Vt = np.linalg.svd(M, full_matrices=False)
    us = []
    for j in range(2):
        B = Vt[j].reshape(PSF_SIZE, PSF_SIZE)
        ev, evec = np.linalg.eigh(B)
        us.append(evec[:, np.argmax(np.abs(ev))])
    us = np.array(us)
    co = None
    for _ in range(300):
        G = np.stack([np.outer(u, u).reshape(-1) for u in us])
        F = np.linalg.solve(G @ G.T, G @ M.T)
        co = [np.polyfit(cg, F[j], 1) for j in range(2)]
        F = np.stack([np.polyval(co[j], cg) for j in range(2)])
        for j in range(2):
            Rj = M - F.T @ G + np.outer(F[j], G[j])
            A = (F[j][:, None] * Rj).sum(0).reshape(PSF_SIZE, PSF_SIZE)
            A = (A + A.T) / (2.0 * (F[j] ** 2).sum())
            ev, evec = np.linalg.eigh(A)
            us[j] = evec[:, np.argmax(np.abs(ev))]
            G[j] = np.outer(us[j], us[j]).reshape(-1)
    return [us[0], us[1]], co


def _band_tables(taps_list, c0s):
    """Compact band tables [4 (q), 128, R*160] bf16:
    tab[q][p, r*160 + (c - c0s[q])] = taps_r[15 + c - (128q+p)]."""
    tab = np.zeros((NCHUNK, 128, R * BW), dtype=np.float64)
    for r, taps in enumerate(taps_list):
        for q in range(NCHUNK):
            c0 = c0s[q]
            for p in range(128):
                row = 128 * q + p
                j0 = max(c0, row - 15, 0)
                j1 = min(c0 + BW, row + 16, H)
                if j1 > j0:
                    tab[q, p, r * BW + j0 - c0:
                        r * BW + j1 - c0] = \
                        taps[15 + np.arange(j0, j1) - row]
    return tab.astype(ml_dtypes.bfloat16)


def _opener_table(taps_list):
    """Full-width zero-padded q=0 band rows for the plane-opening
    matmuls: open[p, i, c'] = taps_{OPENER_RS[i]}[15 + c' - p]."""
    open_ = np.zeros((128, len(_OPENER_RS), H), dtype=np.float64)
    for i, r in enumerate(_OPENER_RS):
        taps = taps_list[r]
        for p in range(128):
            j0 = max(0, p - 15)
            j1 = min(H, p + 16)
            open_[p, i, j0:j1] = taps[15 + np.arange(j0, j1) - p]
    return open_.astype(ml_dtypes.bfloat16)


def _build(flin):
    import concourse.bass as bass  # noqa: F401
    import concourse.tile as tile
    from concourse import mybir, bacc

    f32 = mybir.dt.float32
    bf16 = mybir.dt.bfloat16
    AF = mybir.ActivationFunctionType
    ALU = mybir.AluOpType

    nc = bacc.Bacc("TRN2", target_bir_lowering=False, debug=False,
                   disable_frame_to_traceback=True)
    IMG = nc.declare_dram_parameter("image", [3, H, H], bf16, isOutput=False)
    COC = nc.declare_dram_parameter("coc", [H, H], bf16, isOutput=False)
    T1C = nc.declare_dram_parameter("t1c", [NCHUNK, 128, R * BW], bf16,
                                    isOutput=False)
    T2C = nc.declare_dram_parameter("t2c", [NCHUNK, 128, R * BW], bf16,
                                    isOutput=False)
    T2O = nc.declare_dram_parameter("t2open", [128, len(_OPENER_RS), H],
                                    bf16, isOutput=False)
    OUT = nc.declare_dram_parameter("out", [3, H, H], bf16, isOutput=True)

    with tile.TileContext(nc) as tc:
        import contextlib
        ctx = contextlib.ExitStack()
        with ctx:
            tpool = ctx.enter_context(tc.tile_pool(name="ttab", bufs=1))
            cpool = ctx.enter_context(tc.tile_pool(name="coc", bufs=1))
            xpool = ctx.enter_context(tc.tile_pool(name="xin", bufs=1))
            apool = ctx.enter_context(tc.tile_pool(name="abig", bufs=12))
            mpool = ctx.enter_context(tc.tile_pool(name="mtmp", bufs=3))
            accpool = ctx.enter_context(tc.tile_pool(name="acc", bufs=2))
            # 4 two-bank PSUM units keep stage-1 halves and stage-2
            # pairs pipelining without PE stalls on drain latency
            ps = ctx.enter_context(
                tc.tile_pool(name="ps", bufs=4, space="PSUM"))

            t1c = tpool.tile([128, NCHUNK * R * BW], bf16, tag="t1c")
            t2c = tpool.tile([128, NCHUNK * R * BW], bf16, tag="t2c")
            t2open = tpool.tile([128, len(_OPENER_RS) * H], bf16, tag="t2o")
            xs = [xpool.tile([128, NCHUNK * H], bf16, tag=f"xs{ch}",
                             name=f"xs{ch}")
                  for ch in range(3)]
            coc = cpool.tile([128, NCHUNK * H], bf16, tag="coc")
            fmaps = [cpool.tile([128, NCHUNK * H], bf16, tag=f"f{i}",
                                name=f"fmap{i}")
                     for i in range(2)]

            # --- input DMAs: one large transfer per tensor (issue cost
            # ~0.8us each dominates; per-issue latency starved the
            # prologue when loads were chunked).
            def img_load(engine, ch):
                engine.dma_start(
                    xs[ch][:].rearrange("p (q j) -> p q j", q=NCHUNK),
                    IMG[ch].rearrange("(q p) j -> p q j", p=128))

            nc.sync.dma_start(
                t1c[:].rearrange("p (q j) -> p q j", q=NCHUNK),
                T1C.rearrange("q p j -> p q j"))
            img_load(nc.scalar, 0)
            img_load(nc.sync, 1)
            nc.sync.dma_start(
                t2c[:].rearrange("p (q j) -> p q j", q=NCHUNK),
                T2C.rearrange("q p j -> p q j"))
            nc.gpsimd.dma_start(t2open[:],
                                T2O.rearrange("p i j -> p (i j)"))
            nc.gpsimd.dma_start(
                coc[:].rearrange("p (q j) -> p q j", q=NCHUNK),
                COC.rearrange("(q p) j -> p q j", p=128))
            img_load(nc.gpsimd, 2)

            # per-pixel coefficient maps f_i = a_i * coc + b_i (linear
            # fit coefficients are compile-time immediates)
            for i in range(2):
                nc.vector.tensor_scalar(
                    fmaps[i][:], coc[:], float(flin[i][0]),
                    float(flin[i][1]), ALU.mult, ALU.add)

            abs_ = {}

            def emit_s1_half(ch, mt, hf, drain_dve):
                """Column-conv half block: banks (2hf, 2hf+1) of the
                128-aligned band layout on one 2-bank unit.  Chunk q's
                main writes bank q cols [16,144); cross-chunk seams
                accumulate into the neighbor bank (N=45); one 4D drain
                copies rows c = [256hf, 256hf+256) into ab."""
                P = ps.tile([128, 1024], f32, tag="ps",
                            name=f"b1_{ch}_{mt}_{hf}")
                qs = (2 * hf, 2 * hf + 1)

                def pview(q, j0, j1):
                    off = (q - qs[0]) * 512
                    return P[:, off:off + R * BW].rearrange(
                        "p (r j) -> p r j", r=R)[:, :, j0:j1]

                def tview(q, j0, j1):
                    return t1c[:, q * R * BW:(q + 1) * R * BW].rearrange(
                        "p (r j) -> p r j", r=R)[:, :, j0:j1]

                def lhs(q):
                    return xs[ch][:, q * H + 128 * mt:
                                  q * H + 128 * mt + 128]

                # mains first (start=True resets [16,144)), then seams
                for q in qs:
                    nc.tensor.matmul(pview(q, 16, 144), lhs(q),
                                     tview(q, 16, 144),
                                     start=True, stop=False,
                                     skip_group_check=True)
                # seams (chunk s -> bank b): s=b+1 lands in bank cols
                # [129,144) from its table cols [1,16); s=b-1 lands in
                # [16,31) from cols [144,159)
                if hf == 0:
                    seams = [(0, 1, False), (1, 0, True), (2, 1, True)]
                else:
                    seams = [(1, 2, False), (3, 2, True), (2, 3, True)]
                for s, b, stop in seams:
                    j0, sj0 = (129, 1) if s > b else (16, 144)
                    nc.tensor.matmul(pview(b, j0, j0 + 15), lhs(s),
                                     tview(s, sj0, sj0 + 15),
                                     start=False, stop=stop,
                                     skip_group_check=True)
                if (ch, mt) not in abs_:
                    abs_[(ch, mt)] = apool.tile([128, R * H], bf16,
                                                tag="ab",
                                                name=f"ab{ch}_{mt}")
                ab = abs_[(ch, mt)]
                src = P[:].rearrange("p (q x) -> p q x", q=2)[
                    :, :, 0:R * BW].rearrange(
                    "p q (r j) -> p r q j", r=R)[:, :, :, 16:144]
                dst = ab[:].rearrange("p (r qq j) -> p r qq j",
                                      qq=NCHUNK, j=128)[:, :, qs[0]:qs[1] + 1]
                if drain_dve:
                    nc.vector.tensor_copy(dst, src)
                else:
                    nc.scalar.activation(dst, src, AF.Copy)
                return ab

            def emit_s2_mm(ch, pi, pl, Z):
                """Row-conv matmuls for plane pl of row tiles
                (2pi, 2pi+1) into the 2-bank unit Z (cols [mi*512..])."""
                abig = [abs_[(ch, mt)] for mt in range(NCHUNK)]
                rs = _PLANE_RS[pl]
                for mi in range(2):
                    mtc = 2 * pi + mi
                    off = mi * 512
                    mms = [(q2, r) for q2 in range(NCHUNK) for r in rs]
                    last = mms[-1]
                    for q2, r in mms:
                        lhsT = abig[q2][:, r * H + 128 * mtc:
                                        r * H + 128 * mtc + 128]
                        if q2 == 0 and r == rs[0]:
                            oi = _OPENER_RS.index(r)
                            rhs = t2open[:, oi * H:(oi + 1) * H]
                            nc.tensor.matmul(
                                Z[:, off:off + 512], lhsT, rhs,
                                start=True, stop=((q2, r) == last),
                                skip_group_check=True)
                        else:
                            rhs = t2c[:, q2 * R * BW + r * BW:
                                      q2 * R * BW + (r + 1) * BW]
                            c0 = _C0_S2[q2]
                            nc.tensor.matmul(
                                Z[:, off + c0:off + c0 + BW], lhsT,
                                rhs, start=False,
                                stop=((q2, r) == last),
                                skip_group_check=True)

            s2state = {}

            def emit_s2_B(ch, pi):
                """Plane B (term 2) for a pair on a fresh 2-bank unit;
                the pair-wide m = B .* f2map frees the unit for A."""
                Z = ps.tile([128, 1024], f32, tag="ps",
                            name=f"z_{ch}_{pi}")
                emit_s2_mm(ch, pi, 1, Z)
                fsl = fmaps[1][:, 1024 * pi:1024 * (pi + 1)]
                m = mpool.tile([128, 1024], bf16, tag="m")
                nc.vector.tensor_tensor(m[:], Z[:], fsl, ALU.mult)
                s2state[(ch, pi)] = (Z, m)

            def emit_s2_A(ch, pi):
                """Plane A (term 1) reuses the pair's unit (openers
                reset it); acc = A.*f1map + m; one 256-row DMA."""
                Z, m = s2state.pop((ch, pi))
                emit_s2_mm(ch, pi, 0, Z)
                fsl = fmaps[0][:, 1024 * pi:1024 * (pi + 1)]
                m2 = mpool.tile([128, 1024], bf16, tag="m")
                nc.vector.tensor_tensor(m2[:], Z[:], fsl, ALU.mult)
                acc = accpool.tile([128, 1024], bf16, tag="acc")
                nc.vector.tensor_tensor(acc[:], m2[:], m[:], ALU.add)
                nc.gpsimd.dma_start(
                    OUT[ch][256 * pi:256 * (pi + 1), :].rearrange(
                        "(m p) j -> p m j", p=128),
                    acc[:].rearrange("p (m j) -> p m j", m=2))

            # schedule: ch0 stage-1 prologue with drains alternating
            # ACT/DVE for fast unit turnover; then each stage-2 pair's
            # B and A matmul groups are separated by four stage-1
            # halves of the next channel so the PE never waits on the
            # m = B.*coc multiply that frees the pair's PSUM unit.
            def h(ch, mt, hf, dve=False):
                emit_s1_half(ch, mt, hf, dve)

            for mt in range(NCHUNK):
                h(0, mt, 0, dve=False)
                h(0, mt, 1, dve=True)
            emit_s2_B(0, 0)
            h(1, 0, 0); h(1, 0, 1); h(1, 1, 0); h(1, 1, 1)
            emit_s2_A(0, 0)
            emit_s2_B(0, 1)
            h(1, 2, 0); h(1, 2, 1); h(1, 3, 0); h(1, 3, 1)
            emit_s2_A(0, 1)
            emit_s2_B(1, 0)
            h(2, 0, 0); h(2, 0, 1); h(2, 1, 0); h(2, 1, 1)
            emit_s2_A(1, 0)
            emit_s2_B(1, 1)
            h(2, 2, 0); h(2, 2, 1); h(2, 3, 0); h(2, 3, 1)
            emit_s2_A(1, 1)
            emit_s2_B(2, 0)
            emit_s2_B(2, 1)
            emit_s2_A(2, 0)
            emit_s2_A(2, 1)

    nc.compile()
    return nc


_CACHE = {}


def _get_fit(ws, bs):
    key = (float(ws), float(bs))
    if key not in _CACHE:
        taps, lin = _filters(*key)
        nc = _build(lin)
        _CACHE[key] = (nc, (_band_tables(taps, _C0_S1),
                            _band_tables(taps, _C0_S2),
                            _opener_table(taps)))
    return _CACHE[key]


def _get_prog(ws=0.3, bs=0.5):
    return _get_fit(ws, bs)[0]


def make_in_maps(image, coc_map, w_sigma, b_sigma):
    bf = ml_dtypes.bfloat16
    _, (tab1, tab2, t2open) = _get_fit(
        float(np.asarray(w_sigma).reshape(-1)[0]),
        float(np.asarray(b_sigma).reshape(-1)[0]))
    image = np.asarray(image)
    coc_map = np.asarray(coc_map)
    in_maps = []
    for b in range(image.shape[0]):
        in_maps.append({
            "image": np.ascontiguousarray(image[b].astype(bf)),
            "coc": np.ascontiguousarray(coc_map[b, 0].astype(bf)),
            "t1c": tab1,
            "t2c": tab2,
            "t2open": t2open,
        })
    return in_maps


def kernel(image, coc_map, psf_params, w_sigma, b_sigma):
    from concourse.bass_utils import run_bass_kernel_spmd

    B = image.shape[0]
    assert image.shape == (8, 3, H, H)
    nc, _tabs = _get_fit(
        float(np.asarray(w_sigma).reshape(-1)[0]),
        float(np.asarray(b_sigma).reshape(-1)[0]))
    in_maps = make_in_maps(image, coc_map, w_sigma, b_sigma)
    res = run_bass_kernel_spmd(nc, in_maps, core_ids=list(range(B)))
    out = np.stack([res.results[b]["out"] for b in range(B)], axis=0)
    return np.ascontiguousarray(out).astype(np.float32)


if __name__ == "__main__":
    _get_prog()
    print("build ok")
